# revision 1
# baseline (speedup 1.0000x reference)
"""ArcLengthLoss distributed Bass kernel for 8 TRN2 NeuronCores (v3).

Reference computation:
    s = output[:, :, 0]                               # [32, 153]
    A = s[:, a1] - s[:, a2]; a_term = exp(A.mean(1))  # [32]
    b1 = s[:, direct]                                 # [32, NC]
    b2 = sum_l mask(l<seg_len) * s[:, pad_idx[:, l]]  # [32, NC]
    loss = (a_term + |b1-b2|.mean(1)).mean()

The per-combo gather/sum is a matmul against a signed indicator matrix
W[sec, combo] built directly in [section-row, combo] layout.  Sections are
ranked by usage on the host:
  rows 0..95    mid-popularity sections -> GPSIMD local_scatter from a
                host-built CSR (idx|val int16, preloaded to SBUF once)
  rows 96..127  32 most-used sections (~70% of refs) -> dense bf16 strip
                DMA'd straight into the W tile (signs baked by host)
  rows 128+     25 rare sections -> K=32-worth second matmul on 2 "hi"
                tiles per core holding every combo that references them
W production is split between gpsimd (scatter port ~165 GB/s) and plain
HBM DMA: most tiles ship as host-prebuilt dense bf16, one mega-DMA per
consecutive dense run.  K=128 matmuls use 4-way PE tile_position packing;
abs+sum drains alternate Scalar ACT and a 3-pass Vector sequence.
Combos the scatter cannot express (duplicate targets, capacity overflow)
are computed exactly on the host (0 rows for the reference tables).
"""
import sys

if "/opt/trn_rl_repo" not in sys.path:
    sys.path.insert(0, "/opt/trn_rl_repo")

import numpy as np
import ml_dtypes

import concourse.bass as bass  # noqa: F401
import concourse.bacc as bacc
import concourse.tile as tile
from concourse.tile import add_dep_helper
from concourse import mybir
from concourse.bass_utils import run_bass_kernel_spmd

# ---- problem constants ----
B = 32
S = 153
L = 17
NA = 136
CORES = 8
TILE = 1024
T = 32                    # tiles per core
PERCORE = T * TILE
NTOT = PERCORE * CORES

NBM = 32                  # dense-strip rows (partitions 96..127)
NSC = 96                  # scatter rows (partitions 0..95)
NI = 176                  # static num_idxs per scatter row
NIH = 64                  # static num_idxs per hi row
TD = 27                   # dense (DMA-streamed) tiles per core
N_HI_TILES = 2
MAXRUN = 7                # max dense tiles per mega-DMA


def _set_mode(td):
    global TD, TG, _dense_flags, DENSE_TILES, GP_TILES, HI_TILES, HI_CAP
    global _GP_POS, _DN_POS, _HI_POS, DENSE_RUNS
    TD = td
    TG = T - TD
    _dense_flags = [(t * TD) // T != ((t + 1) * TD) // T for t in range(T)]
    DENSE_TILES = [t for t in range(T) if _dense_flags[t]]
    GP_TILES = [t for t in range(T) if not _dense_flags[t]]
    HI_TILES = GP_TILES[-N_HI_TILES:] if TG >= N_HI_TILES else []
    HI_CAP = len(HI_TILES) * TILE * CORES
    _GP_POS = {t: i for i, t in enumerate(GP_TILES)}
    _DN_POS = {t: i for i, t in enumerate(DENSE_TILES)}
    _HI_POS = {t: i for i, t in enumerate(HI_TILES)}
    # consecutive dense runs (t_start -> run length), split at MAXRUN;
    # the very first chunk is capped at 2 tiles so matmuls start early
    DENSE_RUNS = {}
    run = []
    first = True
    for t in range(T + 1):
        cap = 2 if first else MAXRUN
        if t < T and _dense_flags[t]:
            run.append(t)
            if len(run) == cap:
                DENSE_RUNS[run[0]] = len(run)
                run = []
                first = False
        else:
            if run:
                DENSE_RUNS[run[0]] = len(run)
                first = False
            run = []


_set_mode(TD)

_DT = mybir.dt
_CACHE = {}
DRAIN_MIXED = False       # 3-pass Vector drains are slower than Scalar ACT


def build_nc():
    nc = bacc.Bacc("TRN2", target_bir_lowering=False, debug=False,
                   num_devices=CORES)

    s_d = nc.dram_tensor("sTa", [160, B], _DT.float32, kind="ExternalInput")
    a12_d = nc.dram_tensor("a12", [1, 2 * NA], _DT.int32, kind="ExternalInput")
    wd_d = nc.dram_tensor("WD", [TD, 128, TILE], _DT.int16,
                          kind="ExternalInput")
    if TG:
        bms_d = nc.dram_tensor("BMS", [TG, NBM, TILE], _DT.int16,
                               kind="ExternalInput")
        ixd_d = nc.dram_tensor("IXD", [TG, NSC, 2 * NI], _DT.int16,
                               kind="ExternalInput")
    if HI_TILES:
        hxd_d = nc.dram_tensor("HXD", [N_HI_TILES, 32, 2 * NIH], _DT.int16,
                               kind="ExternalInput")
    o_d = nc.dram_tensor("outv", [128, 2], _DT.float32, kind="ExternalOutput")

    TT = mybir.AluOpType

    with tile.TileContext(nc) as tc:
        with (
            tc.tile_pool(name="const", bufs=1) as cpool,
            tc.tile_pool(name="wts", bufs=2) as wpool,
            tc.tile_pool(name="gwts", bufs=5) as gpool,
            tc.tile_pool(name="drain", bufs=3) as dpool,
            tc.tile_pool(name="psum", bufs=6, space="PSUM") as ppool,
            tc.tile_pool(name="psumA", bufs=1, space="PSUM") as papool,
        ):
            # ---- input DMAs (small constants + full CSR preload)
            sTl_f = cpool.tile([128, B], _DT.float32)
            nc.scalar.dma_start(sTl_f[:], s_d.ap()[0:128])
            sTh_f = cpool.tile([32, B], _DT.float32)
            nc.scalar.dma_start(sTh_f[:], s_d.ap()[128:160])
            a12r = cpool.tile([1, 2 * NA], _DT.int32)
            nc.scalar.dma_start(a12r[:], a12_d.ap())
            if HI_TILES:
                hxd_sb = cpool.tile([32, N_HI_TILES * 2 * NIH], _DT.int16)
                nc.sync.dma_start(
                    hxd_sb[:].rearrange("p (g c) -> p g c", g=N_HI_TILES),
                    hxd_d.ap().rearrange("g p c -> p g c"))

            # ---- converts (vector)
            sT_lo = cpool.tile([128, B], _DT.bfloat16)
            nc.vector.tensor_copy(sT_lo[:], sTl_f[:])
            sT_hi = cpool.tile([128, B], _DT.bfloat16)
            nc.vector.memset(sT_hi[:], 0.0)
            nc.vector.tensor_copy(sT_hi[0:32, :], sTh_f[:])
            a12b16 = cpool.tile([1, 2 * NA], _DT.bfloat16)
            nc.vector.tensor_copy(a12b16[:], a12r[:])
            ones1 = cpool.tile([1, 128], _DT.bfloat16)
            nc.vector.memset(ones1[:], 1.0)

            # ---- gpsimd setup: std-lib ops first, then all local_scatters
            _gs = []
            iota_c = cpool.tile([128, 1], _DT.float32)
            _gs.append(nc.gpsimd.iota(iota_c[:], pattern=[[0, 1]], base=0,
                                      channel_multiplier=1,
                                      allow_small_or_imprecise_dtypes=True))
            iota_ch = cpool.tile([32, 1], _DT.float32)
            _gs.append(nc.gpsimd.iota(iota_ch[:], pattern=[[0, 1]], base=128,
                                      channel_multiplier=1,
                                      allow_small_or_imprecise_dtypes=True))
            wdum = cpool.tile([16, 2], _DT.bfloat16)
            idum = cpool.tile([16, 2], _DT.int16)
            ddum = cpool.tile([16, 2], _DT.bfloat16)
            _gs.append(nc.gpsimd.iota(idum[:], pattern=[[1, 2]], base=0,
                                      channel_multiplier=0))
            _gs.append(nc.gpsimd.memset(ddum[:], 0.0))
            warm = nc.gpsimd.local_scatter(wdum[:], ddum[:], idum[:],
                                           channels=16, num_elems=2,
                                           num_idxs=2)
            for _i in _gs:
                add_dep_helper(warm.ins, _i.ins, sync=False,
                               reason="gpsimd lib grouping")
            # hi strips, built once (rows 32.. zeroed: K=128 matmuls stay
            # uniform inside the accumulation groups)
            hi_w = []
            for i in range(len(HI_TILES)):
                hw = cpool.tile([128, TILE], _DT.bfloat16, tag=f"hi_w{i}")
                nc.vector.memset(hw[:], 0.0)
                base = i * 2 * NIH
                nc.gpsimd.local_scatter(
                    hw[0:32, :],
                    hxd_sb[:, base + NIH:base + 2 * NIH].bitcast(_DT.bfloat16),
                    hxd_sb[:, base:base + NIH],
                    channels=32, num_elems=TILE, num_idxs=NIH)
                hi_w.append(hw)

            # ---- pipeline state
            bacc_t = cpool.tile([128, T // 2], _DT.float32)
            st = [dict() for _ in range(T)]
            ps_roll = {}
            dq = [nc.sync, nc.scalar]

            def s_load(t):
                if _dense_flags[t]:
                    if t not in DENSE_RUNS:
                        return
                    k = DENSE_RUNS[t]
                    td = _DN_POS[t]
                    mega = wpool.tile([128, k * TILE], _DT.bfloat16,
                                      tag=f"mega{k}", name=f"mega_{t}")
                    eng = dq[td % 2]
                    eng.dma_start(
                        mega[:].rearrange("p (td c) -> p td c", td=k),
                        wd_d.ap()[td:td + k].rearrange(
                            "td p c -> p td c").bitcast(_DT.bfloat16))
                    for i in range(k):
                        st[t + i]["wt"] = mega
                        st[t + i]["off"] = i * TILE
                else:
                    g = _GP_POS[t]
                    w = gpool.tile([128, TILE], _DT.bfloat16, tag="w",
                                   name=f"w_{t}")
                    nc.sync.dma_start(
                        w[96:128, :], bms_d.ap()[g].bitcast(_DT.bfloat16))
                    ixd = gpool.tile([NSC, 2 * NI], _DT.int16, tag="ixd",
                                     name=f"ixd_{t}")
                    nc.scalar.dma_start(ixd[:], ixd_d.ap()[g])
                    st[t]["wt"] = w
                    st[t]["off"] = 0
                    st[t]["ixd"] = ixd

            def s_build(t):
                if _dense_flags[t]:
                    return
                d = st[t]
                ixd = d["ixd"]
                nc.gpsimd.local_scatter(
                    d["wt"][0:NSC, :],
                    ixd[:, NI:2 * NI].bitcast(_DT.bfloat16),
                    ixd[:, 0:NI],
                    channels=NSC, num_elems=TILE, num_idxs=NI)

            def s_mm(t):
                d = st[t]
                if t % 2 == 0:
                    ps_roll["ps"] = ppool.tile([128, 512], _DT.float32,
                                               tag="ps", name=f"ps_{t}")
                psum = ps_roll["ps"]
                d["psum"] = psum
                wt, off = d["wt"], d["off"]
                hi = t in _HI_POS
                for j in range(2):
                    a = (2 * t + j) % 4
                    sub = psum[32 * a:32 * (a + 1), :]
                    nc.tensor.matmul(
                        sub, sT_lo[:], wt[:, off + j * 512:off + (j + 1) * 512],
                        start=True, stop=not hi, skip_group_check=True,
                        tile_position=(0, 32 * a))
                    if hi:
                        hw = hi_w[_HI_POS[t]]
                        nc.tensor.matmul(
                            sub, sT_hi[:], hw[:, j * 512:(j + 1) * 512],
                            start=False, stop=True, skip_group_check=True,
                            tile_position=(0, 32 * a))

            def s_drain(t):
                if t % 2 == 1:
                    psum = st[t]["psum"]
                    col = t // 2
                    if (col % 2 == 0) or not DRAIN_MIXED:
                        trash = dpool.tile([128, 512], _DT.bfloat16,
                                           tag="trash", name=f"trash_{t}")
                        nc.scalar.activation(
                            trash[:], psum[:],
                            mybir.ActivationFunctionType.Abs,
                            accum_out=bacc_t[:, col:col + 1])
                    else:
                        ng = dpool.tile([128, 512], _DT.float32,
                                        tag="ng", name=f"ng_{t}")
                        ab = dpool.tile([128, 512], _DT.float32,
                                        tag="ab", name=f"ab_{t}")
                        nc.vector.tensor_scalar(ng[:], psum[:], -1.0, None,
                                                op0=TT.mult)
                        nc.vector.tensor_tensor(ab[:], psum[:], ng[:],
                                                op=TT.max)
                        nc.vector.tensor_reduce(bacc_t[:, col:col + 1], ab[:],
                                                axis=mybir.AxisListType.X,
                                                op=TT.add)
                st[t] = None

            asum = cpool.tile([B, 1], _DT.float32)

            def emit_a_term():
                psbc = papool.tile([128, 2 * NA], _DT.float32, tag="psbc")
                nc.tensor.matmul(psbc[:], ones1[:], a12b16[:],
                                 start=True, stop=True)
                wa_lo = cpool.tile([128, NA], _DT.bfloat16)
                oh2 = cpool.tile([128, NA], _DT.bfloat16)
                nc.vector.tensor_scalar(wa_lo[:], psbc[:, 0:NA], iota_c[:],
                                        None, op0=TT.is_equal)
                nc.vector.tensor_scalar(oh2[:], psbc[:, NA:], iota_c[:], None,
                                        op0=TT.is_equal)
                nc.vector.tensor_tensor(wa_lo[:], wa_lo[:], oh2[:],
                                        op=TT.subtract)
                wa_hi = cpool.tile([32, NA], _DT.bfloat16)
                oh2h = cpool.tile([32, NA], _DT.bfloat16)
                nc.vector.tensor_scalar(wa_hi[:], psbc[0:32, 0:NA], iota_ch[:],
                                        None, op0=TT.is_equal)
                nc.vector.tensor_scalar(oh2h[:], psbc[0:32, NA:], iota_ch[:],
                                        None, op0=TT.is_equal)
                nc.vector.tensor_tensor(wa_hi[:], wa_hi[:], oh2h[:],
                                        op=TT.subtract)
                psa = papool.tile([B, NA], _DT.float32, tag="psa")
                nc.tensor.matmul(psa[:], sT_lo[:], wa_lo[:],
                                 start=True, stop=False)
                nc.tensor.matmul(psa[:], sT_hi[0:32, :], wa_hi[:],
                                 start=False, stop=True)
                nc.vector.tensor_reduce(asum[:], psa[:],
                                        axis=mybir.AxisListType.X,
                                        op=mybir.AluOpType.add)

            def s_nop(t):
                pass

            stages = [s_load, s_nop, s_build, s_mm, s_drain]
            NS = len(stages)
            for step in range(T + NS - 1):
                for si in reversed(range(NS)):
                    t = step - si
                    if 0 <= t < T:
                        stages[si](t)
                if step == 6:
                    emit_a_term()

            outv = cpool.tile([128, 2], _DT.float32)
            nc.vector.memset(outv[:], 0.0)
            nc.vector.tensor_reduce(outv[:, 0:1], bacc_t[:],
                                    axis=mybir.AxisListType.X,
                                    op=mybir.AluOpType.add)
            nc.vector.tensor_copy(outv[0:B, 1:2], asum[:])
            nc.scalar.dma_start(o_d.ap(), outv[:])

    nc.compile()
    return nc


def prepare(inputs):
    """Host-side prep: rank sections, route combos, build device arrays."""
    s = np.asarray(inputs["output"], np.float32)[:, :, 0]
    a1 = np.asarray(inputs["a1"], np.int64)
    a2 = np.asarray(inputs["a2"], np.int64)
    direct = np.asarray(inputs["direct"], np.int64)
    pad = np.asarray(inputs["pad_idx"], np.int64)
    seg = np.asarray(inputs["seg_len"], np.int64)
    NCv = direct.shape[0]
    lane = np.arange(L)[None, :]
    act = lane < seg[:, None]

    padrefs = np.bincount(pad[act], minlength=S)
    dirrefs = np.bincount(direct, minlength=S)
    usage = padrefs + dirrefs
    order = np.argsort(-usage, kind="stable")
    rank = np.empty(S, np.int64)
    rank[order] = np.arange(S)
    # partition row of each rank: top-32 -> 96..127 (dense strip),
    # next 96 -> 0..95 (scatter), rest -> 128.. (hi strip)
    pr_of_rank = np.concatenate([96 + np.arange(NBM), np.arange(NSC),
                                 128 + np.arange(S - 128)])
    prow = pr_of_rank[rank]                     # section -> partition row

    # collision rows (duplicate scatter targets) -> host
    a_ = np.where(act, pad, 2000 + lane)
    tcat = np.concatenate([np.where(direct < S, direct, 3000)[:, None], a_], 1)
    tcat.sort(axis=1)
    host = (tcat[:, 1:] == tcat[:, :-1]).any(1)

    hi_sec = np.zeros(S, bool)
    hi_sec[order[128:]] = True
    has_hi = hi_sec[direct] | (hi_sec[pad] & act).any(1)
    hi_mask = has_hi & ~host
    hi_idx = np.flatnonzero(hi_mask)
    if hi_idx.size > HI_CAP:
        host[hi_idx[HI_CAP:]] = True
        hi_idx = hi_idx[:HI_CAP]
    lo_idx = np.flatnonzero(~has_hi & ~host)

    rng = np.random.default_rng(9)
    core_of = np.full(NCv, -1, np.int64)
    tile_of = np.full(NCv, -1, np.int64)
    col_of = np.full(NCv, -1, np.int64)

    h = rng.permutation(hi_idx)
    hc = np.arange(h.size)
    core_of[h] = hc % CORES
    ht = hc // CORES
    if h.size:
        tile_of[h] = np.asarray(HI_TILES)[ht % N_HI_TILES]
        col_of[h] = ht // N_HI_TILES

    p = rng.permutation(lo_idx)
    used = np.zeros((CORES, T), np.int64)
    if h.size:
        np.add.at(used, (core_of[h], tile_of[h]), 1)
    free_cols = TILE - used
    cc, tt = np.meshgrid(np.arange(CORES), np.arange(T), indexing="ij")
    reps = free_cols.ravel()
    slot_core = np.repeat(cc.ravel(), reps)
    slot_tile = np.repeat(tt.ravel(), reps)
    slot_col = (np.arange(reps.sum()) -
                np.repeat(np.cumsum(reps) - reps, reps) +
                np.repeat(used.ravel(), reps))
    assert p.size <= slot_core.size, "combo overflow"
    n = p.size
    core_of[p] = slot_core[:n]
    tile_of[p] = slot_tile[:n]
    col_of[p] = slot_col[:n]

    is_dense_tile = np.asarray(_dense_flags)

    def build_entries():
        dev = np.flatnonzero(core_of >= 0)
        pr, pl = np.nonzero(act[dev])
        e_combo = np.concatenate([dev[pr], dev])
        e_row = np.concatenate([prow[pad[dev[pr], pl]], prow[direct[dev]]])
        e_val = np.concatenate([np.full(pr.size, -1.0, np.float32),
                                np.full(dev.size, 1.0, np.float32)])
        return (e_combo, e_row, e_val, core_of[e_combo], tile_of[e_combo],
                col_of[e_combo])

    e_combo, e_row, e_val, e_core, e_tile, e_col = build_entries()

    gp_pos_arr = np.full(T, -1, np.int64)
    for i, t_ in enumerate(GP_TILES):
        gp_pos_arr[t_] = i
    dn_pos_arr = np.full(T, -1, np.int64)
    for i, t_ in enumerate(DENSE_TILES):
        dn_pos_arr[t_] = i
    hi_pos_arr = np.full(T, -1, np.int64)
    for i, t_ in enumerate(HI_TILES):
        hi_pos_arr[t_] = i

    def prune(sel, cap, keyfn):
        nonlocal e_combo, e_row, e_val, e_core, e_tile, e_col, host
        idxs = np.flatnonzero(sel)
        if idxs.size == 0:
            return False
        key = keyfn(idxs)
        sort = np.argsort(key, kind="stable")
        ks = key[sort]
        first = np.r_[0, np.flatnonzero(np.diff(ks)) + 1]
        counts = np.diff(np.r_[first, ks.size])
        pos = np.arange(ks.size) - np.repeat(first, counts)
        bad = sort[pos >= cap]
        if bad.size == 0:
            return False
        bad_combos = np.unique(e_combo[idxs[bad]])
        host[bad_combos] = True
        core_of[bad_combos] = -1
        keep = core_of[e_combo] >= 0
        e_combo, e_row, e_val = e_combo[keep], e_row[keep], e_val[keep]
        e_core, e_tile, e_col = e_core[keep], e_tile[keep], e_col[keep]
        return True

    for _ in range(3):
        c1 = prune((e_row < NSC) & ~is_dense_tile[e_tile], NI,
                   lambda ii: (e_core[ii] * TG + gp_pos_arr[e_tile[ii]]) * NSC
                   + e_row[ii])
        c2 = prune(e_row >= 128, NIH,
                   lambda ii: (e_core[ii] * N_HI_TILES +
                               hi_pos_arr[e_tile[ii]]) * 32 +
                   (e_row[ii] - 128))
        if not (c1 or c2):
            break

    dense_e = is_dense_tile[e_tile]
    WD = np.zeros((CORES, TD, 128, TILE), ml_dtypes.bfloat16)
    de = np.flatnonzero(dense_e)
    WD[e_core[de], dn_pos_arr[e_tile[de]], e_row[de], e_col[de]] = \
        e_val[de].astype(ml_dtypes.bfloat16)
    WD = WD.view(np.int16)

    # dense strip rows (96..127) of gpsimd tiles, signs baked
    bm_e = (~dense_e) & (e_row >= NSC) & (e_row < 128)
    BMS = np.zeros((CORES, TG, NBM, TILE), ml_dtypes.bfloat16)
    be = np.flatnonzero(bm_e)
    BMS[e_core[be], gp_pos_arr[e_tile[be]], e_row[be] - NSC, e_col[be]] = \
        e_val[be].astype(ml_dtypes.bfloat16)
    BMS = BMS.view(np.int16)

    # scatter rows: combined idx|val int16 [CORES, TG, 96, 2*NI]
    sc = (~dense_e) & (e_row < NSC)
    se = np.flatnonzero(sc)
    skey = (e_core[se] * TG + gp_pos_arr[e_tile[se]]) * NSC + e_row[se]
    sort = np.argsort(skey, kind="stable")
    ks = skey[sort]
    _, first_idx, counts = np.unique(ks, return_index=True, return_counts=True)
    pos = np.arange(ks.size) - np.repeat(first_idx, counts)
    IXD = np.full((CORES, TG, NSC, 2 * NI), -1, np.int16)
    IXD[:, :, :, NI:] = 0
    ses = se[sort]
    IXD.reshape(-1, 2 * NI)[ks, pos] = e_col[ses].astype(np.int16)
    IXD.reshape(-1, 2 * NI)[ks, NI + pos] = \
        e_val[ses].astype(ml_dtypes.bfloat16).view(np.int16)

    # hi strip: combined idx|val [CORES, N_HI_TILES, 32, 2*NIH]
    HXD = np.full((CORES, N_HI_TILES, 32, 2 * NIH), -1, np.int16)
    HXD[:, :, :, NIH:] = 0
    he = np.flatnonzero(e_row >= 128)
    if he.size:
        hkey = ((e_core[he] * N_HI_TILES + hi_pos_arr[e_tile[he]]) * 32 +
                (e_row[he] - 128))
        sort = np.argsort(hkey, kind="stable")
        ks = hkey[sort]
        _, first_idx, counts = np.unique(ks, return_index=True,
                                         return_counts=True)
        pos = np.arange(ks.size) - np.repeat(first_idx, counts)
        hes = he[sort]
        HXD.reshape(-1, 2 * NIH)[ks, pos] = e_col[hes].astype(np.int16)
        HXD.reshape(-1, 2 * NIH)[ks, NIH + pos] = \
            e_val[hes].astype(ml_dtypes.bfloat16).view(np.int16)

    sTa = np.zeros((160, B), np.float32)
    sTa[prow] = s.T
    a12 = np.concatenate([prow[a1], prow[a2]]).astype(np.int32).reshape(
        1, 2 * NA)

    hs = np.flatnonzero(host)
    host_abs = 0.0
    if hs.size:
        m = act[hs].astype(np.float32)
        b2 = np.einsum("bnl,nl->bn", s[:, pad[hs]], m)
        b1 = s[:, direct[hs]]
        host_abs = float(np.abs(b1 - b2).sum())

    in_maps = []
    for c in range(CORES):
        m = {"sTa": sTa, "a12": a12, "WD": WD[c]}
        if TG:
            m.update({"BMS": BMS[c], "IXD": IXD[c]})
        if HI_TILES:
            m.update({"HXD": HXD[c]})
        in_maps.append(m)
    return in_maps, dict(NCv=NCv, host_abs=host_abs, n_host=int(hs.size))


def combine(outs, meta):
    total_abs = meta["host_abs"] + sum(float(outs[i]["outv"][:, 0].sum())
                                       for i in range(CORES))
    mean_a = float(np.exp(outs[0]["outv"][0:B, 1] / NA).mean())
    val = mean_a + total_abs / (B * meta["NCv"])
    return np.asarray(val, dtype=np.float32)


def get_nc():
    if "nc" not in _CACHE:
        _CACHE["nc"] = build_nc()
    return _CACHE["nc"]


def kernel(**inputs) -> np.ndarray:
    in_maps, meta = prepare(inputs)
    res = run_bass_kernel_spmd(get_nc(), in_maps, core_ids=list(range(CORES)))
    return combine(res.results, meta)



# revision 13
# speedup vs baseline: 1.2494x; 1.2494x over previous
"""ArcLengthLoss distributed Bass kernel for 8 TRN2 NeuronCores (v5).

Reference computation:
    s = output[:, :, 0]                               # [32, 153]
    A = s[:, a1] - s[:, a2]; a_term = exp(A.mean(1))  # [32]
    b1 = s[:, direct]                                 # [32, NC]
    b2 = sum_l mask(l<seg_len) * s[:, pad_idx[:, l]]  # [32, NC]
    loss = (a_term + |b1-b2|.mean(1)).mean()

The per-combo gather/sum is a matmul against a signed indicator matrix
W[sec, combo], shipped fully dense in fp8 (0/±1 exact in e4m3 — half the
HBM bytes of bf16).  K=128 usage-ranked sections ride the partition dim;
matmuls use 4-way tile_position packing (DoubleRow fp8 was rejected: the
ISA pins DoubleRow outputs to PSUM partition 0, killing the packing).
The 25 rarest sections live in a zero-padded K=128 "hi" strip
chain-accumulated on two designated tiles.  W is stored partition-major
in DRAM (4KB contiguous per partition per mega) and streamed in runs of
4 tiles alternating between the sync HWDGE queue and the gpsimd SWDGE
queue, so the scalar engine only drains.  All small inputs ride in two
DMAs: SP (s in both layouts) and WA (host-prebuilt A-term one-hots —
no device iota/is_equal at all).  PSUM drains alternate scalar ACT Abs
(+accum) with a two-pass Vector path (negate-to-bf16, then all-SBUF 2x
scalar_tensor_tensor max with accumulate).  tensor_tensor_reduce is
avoided: it crashes the exec unit on this runtime.  Combos the dense
build cannot express exactly (duplicate targets -> |entry| could exceed
fp8 integer range) are computed on the host (0 for reference tables).
"""
import sys

if "/opt/trn_rl_repo" not in sys.path:
    sys.path.insert(0, "/opt/trn_rl_repo")

import numpy as np
import ml_dtypes

import concourse.bass as bass  # noqa: F401
import concourse.bacc as bacc
import concourse.tile as tile
from concourse import mybir
from concourse.bass_utils import run_bass_kernel_spmd

# ---- problem constants ----
B = 32
S = 153
L = 17
NA = 136
CORES = 8
TILE = 1024
T = 32                    # tiles per core
PERCORE = T * TILE
NTOT = PERCORE * CORES

N_HI_TILES = 2
HI_TILES = [30, 31]
HI_CAP = N_HI_TILES * TILE * CORES
# mega-DMA runs (t_start -> length); two 1-tile runs first (parallel on
# both queues) so matmuls start early
DENSE_RUNS = {0: 1, 1: 1, 2: 4, 6: 4, 10: 4, 14: 4, 18: 4, 22: 4, 26: 4,
              30: 2}
SCALAR_DRAINS = (0, 1, 2, 3, 7)

_DT = mybir.dt
_CACHE = {}


def build_nc():
    nc = bacc.Bacc("TRN2", target_bir_lowering=False, debug=False,
                   num_devices=CORES)

    sp_d = nc.dram_tensor("SP", [128, 2 * B], _DT.float32,
                          kind="ExternalInput")
    wa_d = nc.dram_tensor("WA", [128, 2 * NA], _DT.int16,
                          kind="ExternalInput")
    wd_d = nc.dram_tensor("WD", [128, T * TILE], _DT.int8,
                          kind="ExternalInput")
    wh_d = nc.dram_tensor("WH", [32, N_HI_TILES * TILE], _DT.int8,
                          kind="ExternalInput")
    o_d = nc.dram_tensor("outv", [128, 2], _DT.float32, kind="ExternalOutput")

    TT = mybir.AluOpType

    with tile.TileContext(nc) as tc:
        with (
            tc.tile_pool(name="const", bufs=1) as cpool,
            tc.tile_pool(name="wts", bufs=3) as wpool,
            tc.tile_pool(name="drain", bufs=3) as dpool,
            tc.tile_pool(name="psum", bufs=3, space="PSUM") as ppool,
            tc.tile_pool(name="psumA", bufs=1, space="PSUM") as papool,
        ):
            # ---- small input DMAs: SP on scalar (its only queue work),
            # WA + WH on sync after the first mega trigger
            sp_f = cpool.tile([128, 2 * B], _DT.float32)
            nc.scalar.dma_start(sp_f[:], sp_d.ap())
            wa_sb = cpool.tile([128, 2 * NA], _DT.int16)
            wh_sb = cpool.tile([128, N_HI_TILES * TILE], _DT.int8)

            # ---- converts (vector)
            s8_lo = cpool.tile([128, B], _DT.float8e4)
            nc.vector.tensor_copy(s8_lo[:], sp_f[:, 0:B])
            s8_hi = cpool.tile([128, B], _DT.float8e4)
            nc.vector.memset(s8_hi[:], 0.0)
            nc.vector.tensor_copy(s8_hi[0:32, :], sp_f[0:32, B:2 * B])
            sT_lo = cpool.tile([128, B], _DT.bfloat16)
            nc.vector.tensor_copy(sT_lo[:], sp_f[:, 0:B])
            sT_hi = cpool.tile([32, B], _DT.bfloat16)
            nc.vector.tensor_copy(sT_hi[:], sp_f[0:32, B:2 * B])
            nc.vector.memset(wh_sb[:], 0)

            wh8 = wh_sb[:].bitcast(_DT.float8e4).rearrange(
                "p (ht c) -> p ht c", ht=N_HI_TILES)
            wa16 = wa_sb[:].bitcast(_DT.bfloat16)

            # ---- pipeline state
            bacc_t = cpool.tile([128, T // 4], _DT.float32)
            st = [None] * T
            ps_roll = {}
            run_idx = {}
            for i, t0 in enumerate(sorted(DENSE_RUNS)):
                run_idx[t0] = i

            def s_load(t):
                if t not in DENSE_RUNS:
                    return
                k = DENSE_RUNS[t]
                mega = wpool.tile([128, k * TILE], _DT.int8,
                                  tag=f"mega{k}_{run_idx[t] % 2}",
                                  name=f"mega_{t}")
                eng = nc.sync if run_idx[t] % 2 == 0 else nc.gpsimd
                eng.dma_start(mega[:], wd_d.ap()[:, t * TILE:(t + k) * TILE])
                for i in range(k):
                    st[t + i] = (mega, i)
                if t == 0:
                    nc.sync.dma_start(wa_sb[:], wa_d.ap())
                    nc.sync.dma_start(wh_sb[0:32, :], wh_d.ap())

            def s_mm(t):
                a = t % 4
                if a == 0:
                    ps_roll["ps"] = ppool.tile([128, 1024], _DT.float32,
                                               tag="ps", name=f"ps_{t}")
                    ps_roll[t // 4] = ps_roll["ps"]
                psum = ps_roll["ps"]
                mega, i = st[t]
                w8 = mega[:].bitcast(_DT.float8e4)
                off = i * TILE
                hi = t in HI_TILES
                for q in range(2):
                    sub = psum[32 * a:32 * (a + 1), 512 * q:512 * (q + 1)]
                    nc.tensor.matmul(
                        sub, s8_lo[:], w8[:, off + 512 * q:off + 512 * (q + 1)],
                        start=True, stop=not hi,
                        skip_group_check=True, tile_position=(0, 32 * a))
                    if hi:
                        ht = HI_TILES.index(t)
                        nc.tensor.matmul(
                            sub, s8_hi[:],
                            wh8[:, ht, 512 * q:512 * (q + 1)],
                            start=False, stop=True,
                            skip_group_check=True, tile_position=(0, 32 * a))

            def s_drain(t):
                if t % 4 != 3:
                    return
                g = t // 4
                psum = ps_roll.pop(g)
                trash = dpool.tile([128, 1024], _DT.bfloat16,
                                   tag="trash", name=f"trash_{t}")
                if g in SCALAR_DRAINS:
                    nc.scalar.activation(
                        trash[:], psum[:],
                        mybir.ActivationFunctionType.Abs,
                        accum_out=bacc_t[:, g:g + 1])
                else:
                    ng = dpool.tile([128, 1024], _DT.bfloat16,
                                    tag="ng", name=f"ng_{t}")
                    nc.vector.tensor_scalar(ng[:], psum[:], -1.0, None,
                                            op0=TT.mult)
                    nc.vector.scalar_tensor_tensor(
                        trash[:], ng[:], -1.0, ng[:],
                        op0=TT.mult, op1=TT.max,
                        accum_out=bacc_t[:, g:g + 1])
                st[t] = None

            asum = cpool.tile([B, 1], _DT.float32)

            def emit_a_term():
                psa = papool.tile([B, NA], _DT.float32, tag="psa")
                nc.tensor.matmul(psa[:], sT_lo[:], wa16[:, 0:NA],
                                 start=True, stop=False)
                nc.tensor.matmul(psa[:], sT_hi[:], wa16[0:32, NA:2 * NA],
                                 start=False, stop=True)
                nc.vector.tensor_reduce(asum[:], psa[:],
                                        axis=mybir.AxisListType.X,
                                        op=mybir.AluOpType.add)

            def s_nop(t):
                pass

            stages = [s_load, s_nop, s_nop, s_mm, s_drain]
            NS = len(stages)
            for step in range(T + NS - 1):
                for si in reversed(range(NS)):
                    t = step - si
                    if 0 <= t < T:
                        stages[si](t)
                if step == 6:
                    emit_a_term()

            outv = cpool.tile([128, 2], _DT.float32)
            nc.vector.memset(outv[:], 0.0)
            nc.vector.tensor_reduce(outv[:, 0:1], bacc_t[:],
                                    axis=mybir.AxisListType.X,
                                    op=mybir.AluOpType.add)
            nc.vector.tensor_copy(outv[0:B, 1:2], asum[:])
            nc.scalar.dma_start(o_d.ap(), outv[:])

    nc.compile()
    return nc


def prepare(inputs):
    """Host-side prep: rank sections, route combos, build device arrays."""
    s = np.asarray(inputs["output"], np.float32)[:, :, 0]
    a1 = np.asarray(inputs["a1"], np.int64)
    a2 = np.asarray(inputs["a2"], np.int64)
    direct = np.asarray(inputs["direct"], np.int64)
    pad = np.asarray(inputs["pad_idx"], np.int64)
    seg = np.asarray(inputs["seg_len"], np.int64)
    NCv = direct.shape[0]
    lane = np.arange(L)[None, :]
    act = lane < seg[:, None]

    padrefs = np.bincount(pad[act], minlength=S)
    dirrefs = np.bincount(direct, minlength=S)
    usage = padrefs + dirrefs
    order = np.argsort(-usage, kind="stable")
    rank = np.empty(S, np.int64)
    rank[order] = np.arange(S)

    # duplicate targets within a combo could push |W entry| past the fp8
    # exact-integer range -> host
    a_ = np.where(act, pad, 2000 + lane)
    tcat = np.concatenate([np.where(direct < S, direct, 3000)[:, None], a_], 1)
    tcat.sort(axis=1)
    host = (tcat[:, 1:] == tcat[:, :-1]).any(1)

    hi_sec = np.zeros(S, bool)
    hi_sec[order[128:]] = True
    has_hi = hi_sec[direct] | (hi_sec[pad] & act).any(1)
    hi_mask = has_hi & ~host
    hi_idx = np.flatnonzero(hi_mask)
    if hi_idx.size > HI_CAP:
        host[hi_idx[HI_CAP:]] = True
        hi_idx = hi_idx[:HI_CAP]
    lo_idx = np.flatnonzero(~has_hi & ~host)

    rng = np.random.default_rng(9)
    core_of = np.full(NCv, -1, np.int64)
    tile_of = np.full(NCv, -1, np.int64)
    col_of = np.full(NCv, -1, np.int64)

    h = rng.permutation(hi_idx)
    hc = np.arange(h.size)
    core_of[h] = hc % CORES
    ht = hc // CORES
    if h.size:
        tile_of[h] = np.asarray(HI_TILES)[ht % N_HI_TILES]
        col_of[h] = ht // N_HI_TILES

    p = rng.permutation(lo_idx)
    used = np.zeros((CORES, T), np.int64)
    if h.size:
        np.add.at(used, (core_of[h], tile_of[h]), 1)
    free_cols = TILE - used
    cc, tt = np.meshgrid(np.arange(CORES), np.arange(T), indexing="ij")
    reps = free_cols.ravel()
    slot_core = np.repeat(cc.ravel(), reps)
    slot_tile = np.repeat(tt.ravel(), reps)
    slot_col = (np.arange(reps.sum()) -
                np.repeat(np.cumsum(reps) - reps, reps) +
                np.repeat(used.ravel(), reps))
    assert p.size <= slot_core.size, "combo overflow"
    n = p.size
    core_of[p] = slot_core[:n]
    tile_of[p] = slot_tile[:n]
    col_of[p] = slot_col[:n]

    dev = np.flatnonzero(core_of >= 0)
    pr, pl = np.nonzero(act[dev])
    e_combo = np.concatenate([dev[pr], dev])
    e_row = rank[np.concatenate([pad[dev[pr], pl], direct[dev]])]
    e_val = np.concatenate([np.full(pr.size, -1.0, np.float32),
                            np.full(dev.size, 1.0, np.float32)])
    e_core = core_of[e_combo]
    e_tile = tile_of[e_combo]
    e_col = col_of[e_combo]

    hi_pos_arr = np.full(T, -1, np.int64)
    for i, t_ in enumerate(HI_TILES):
        hi_pos_arr[t_] = i

    WDf = np.zeros((CORES, 128, T, TILE), np.float32)
    lo_e = np.flatnonzero(e_row < 128)
    np.add.at(WDf, (e_core[lo_e], e_row[lo_e], e_tile[lo_e], e_col[lo_e]),
              e_val[lo_e])
    WHf = np.zeros((CORES, 32, N_HI_TILES, TILE), np.float32)
    he = np.flatnonzero(e_row >= 128)
    if he.size:
        np.add.at(WHf, (e_core[he], e_row[he] - 128,
                        hi_pos_arr[e_tile[he]], e_col[he]), e_val[he])
    WD = WDf.astype(ml_dtypes.float8_e4m3).view(np.int8).reshape(
        CORES, 128, T * TILE)
    WH = WHf.astype(ml_dtypes.float8_e4m3).view(np.int8).reshape(
        CORES, 32, N_HI_TILES * TILE)

    sTa = np.zeros((160, B), np.float32)
    sTa[rank] = s.T
    SP = np.zeros((128, 2 * B), np.float32)
    SP[:, 0:B] = sTa[0:128]
    SP[0:32, B:2 * B] = sTa[128:160]

    r1 = rank[a1]
    r2 = rank[a2]
    WAf = np.zeros((128, 2 * NA), np.float32)
    i_lo1 = np.flatnonzero(r1 < 128)
    np.add.at(WAf, (r1[i_lo1], i_lo1), 1.0)
    i_lo2 = np.flatnonzero(r2 < 128)
    np.add.at(WAf, (r2[i_lo2], i_lo2), -1.0)
    i_hi1 = np.flatnonzero(r1 >= 128)
    np.add.at(WAf, (r1[i_hi1] - 128, NA + i_hi1), 1.0)
    i_hi2 = np.flatnonzero(r2 >= 128)
    np.add.at(WAf, (r2[i_hi2] - 128, NA + i_hi2), -1.0)
    WA = WAf.astype(ml_dtypes.bfloat16).view(np.int16)

    hs = np.flatnonzero(host)
    host_abs = 0.0
    if hs.size:
        m = act[hs].astype(np.float32)
        b2 = np.einsum("bnl,nl->bn", s[:, pad[hs]], m)
        b1 = s[:, direct[hs]]
        host_abs = float(np.abs(b1 - b2).sum())

    in_maps = []
    for c in range(CORES):
        in_maps.append({"SP": SP, "WA": WA, "WD": WD[c], "WH": WH[c]})
    return in_maps, dict(NCv=NCv, host_abs=host_abs, n_host=int(hs.size))


def combine(outs, meta):
    total_abs = meta["host_abs"] + sum(float(outs[i]["outv"][:, 0].sum())
                                       for i in range(CORES))
    mean_a = float(np.exp(outs[0]["outv"][0:B, 1] / NA).mean())
    val = mean_a + total_abs / (B * meta["NCv"])
    return np.asarray(val, dtype=np.float32)


def get_nc():
    if "nc" not in _CACHE:
        _CACHE["nc"] = build_nc()
    return _CACHE["nc"]


def kernel(**inputs) -> np.ndarray:
    in_maps, meta = prepare(inputs)
    res = run_bass_kernel_spmd(get_nc(), in_maps, core_ids=list(range(CORES)))
    return combine(res.results, meta)
